# revision 2
# baseline (speedup 1.0000x reference)
"""Multi-head attention (B=4, S=2048, E=1024, H=16, D=64) on 8 TRN2 cores.

Sharding: heads 2c, 2c+1 on core c (Megatron-style column-parallel qkv,
row-parallel out-projection; partial outputs summed on host in f32).

v2 vs baseline: all matmul inputs bf16 (PSUM accumulation stays f32, rel err
~4e-3 vs 2e-2 budget) halving input/output DMA, and two structural changes:

1. Row-packed score matmuls: the two heads' K=64 score contractions run
   CONCURRENTLY in the PE array via tile_position=(0,0)/(64,0) (HW-measured
   2.56x over the baseline's zero-padded K=128 trick).
2. The softmax exp stream paces the kernel (ScalarE is the busiest engine:
   256 x exp[128,1024] ~ 290us). Attention runs on 512-wide sq chunks with a
   combined head-pair score tile [128,2,512] (2 PSUM banks) so a 2-deep sc
   pipeline (4 banks) + per-head attnV accumulators (2 banks) + a filler pool
   (2 banks) fit in the 8 PSUM banks. One exp instruction per kt covers both
   heads; the sc pipeline keeps ScalarE saturated while qkv / out-projection /
   v-transpose work is drip-fed to the in-order PE as fine-grained filler
   units (one per kt slot) from a per-batch queue.

attnV keeps the appended ones-column on v producing softmax denominators as
PSUM row 64 (the two heads' v cannot be column-packed: 2x65 outputs > 128
array columns); normalization is reciprocal + partition-broadcast + multiply.
"""
from contextlib import ExitStack

import numpy as np

import concourse.bass as bass
import concourse.mybir as mybir
import concourse.tile as tile
from concourse import bacc
from concourse.bass_utils import run_bass_kernel_spmd
from concourse.masks import make_identity

B, S, E, H, D = 4, 2048, 1024, 16, 64
NCORES = 8
HPC = H // NCORES        # 2 heads per core
F = HPC * D              # 128 local features
M3 = 3 * F               # 384 local qkv rows
BS = B * S               # 8192
KT_E = E // 128          # 8 contraction tiles for projections
KT_S = S // 128          # 16 sk tiles
NG = S // 512            # 4 attention groups (sq chunks) per batch
f32 = mybir.dt.float32
f32r = mybir.dt.float32r
bf16 = mybir.dt.bfloat16
EXP = mybir.ActivationFunctionType.Exp
NPBF16 = mybir.dt.np(bf16)

_prog_cache = {}


def build_program(niter=None, parts="Aao"):
    """niter=None: normal external-I/O program. niter=N: timing variant with
    internal DRAM x/y and the whole body in a device-side For_i loop."""
    key = ("nc", niter, parts)
    if key in _prog_cache:
        return _prog_cache[key]
    nc = bacc.Bacc("TRN2", target_bir_lowering=False)
    if niter is None:
        xT = nc.dram_tensor("xT", [E, BS], bf16, kind="ExternalInput")
        yT = nc.dram_tensor("yT", [E, BS], bf16, kind="ExternalOutput")
    else:
        xT = nc.dram_tensor("xTi", [E, BS], bf16, kind="Internal")
        yT = nc.dram_tensor("yTi", [E, BS], bf16, kind="Internal")
    wq = nc.dram_tensor("wq", [E, M3], bf16, kind="ExternalInput")
    bq = nc.dram_tensor("bq", [128, 3], f32, kind="ExternalInput")
    wo = nc.dram_tensor("wo", [F, E], bf16, kind="ExternalInput")
    bo = nc.dram_tensor("bo", [128, E // 128], f32, kind="ExternalInput")
    if niter is not None:
        tout = nc.dram_tensor("tout", [1, 3], f32, kind="ExternalOutput")

    with tile.TileContext(nc) as tc, ExitStack() as ctx:
        const = ctx.enter_context(tc.tile_pool(name="const", bufs=1))
        xp = ctx.enter_context(tc.tile_pool(name="xp", bufs=2))
        expp = ctx.enter_context(tc.tile_pool(name="expp", bufs=4))
        anp = ctx.enter_context(tc.tile_pool(name="anp", bufs=4))
        asp = ctx.enter_context(tc.tile_pool(name="asp", bufs=2))
        ystp = ctx.enter_context(tc.tile_pool(name="ystp", bufs=3))
        # PSUM (8 banks): pssc 2x[128,2,512] sc-pairs (4), psat 2x[65,512]
        # attnV accumulators (2), psfl 2x[128,512] filler slots (2).
        pssc = ctx.enter_context(tc.tile_pool(name="pssc", bufs=2, space="PSUM"))
        psat = ctx.enter_context(tc.tile_pool(name="psat", bufs=2, space="PSUM"))
        psfl = ctx.enter_context(tc.tile_pool(name="psfl", bufs=2, space="PSUM"))

        wq_sb = const.tile([128, KT_E, M3], bf16)
        nc.gpsimd.dma_start(out=wq_sb, in_=wq.rearrange("(kt p) m -> p kt m", p=128))
        wo_sb = const.tile([F, E], bf16)
        nc.gpsimd.dma_start(out=wo_sb, in_=wo[:, :])
        bq_sb = const.tile([128, 3], f32)
        nc.gpsimd.dma_start(out=bq_sb, in_=bq[:, :])
        bo_sb = const.tile([128, E // 128], f32)
        nc.gpsimd.dma_start(out=bo_sb, in_=bo[:, :])
        id_f32 = const.tile([128, 128], f32)
        make_identity(nc, id_f32)
        id_sb = const.tile([128, 128], f32r)
        nc.vector.tensor_copy(id_sb, id_f32)

        xT_r = xT.rearrange("(kt p) n -> p kt n", p=128)

        # persistent double-buffered qkv storage, slot b%2.
        q_st = const.tile([128, 2, S], bf16, name="q_st")
        k_st = const.tile([128, 2, S], bf16, name="k_st")
        v_st = const.tile([128, 2, S], f32r, name="v_st")
        # vk: [part, slot, kt, head, 65] bf16; col 64 = ones (denominator).
        vk_st = const.tile([128, 2, KT_S, HPC, 65], bf16, name="vk_st")
        nc.vector.memset(vk_st[:, :, :, :, 64:65], 1.0)

        def body():
            qkvt = {}   # batch -> [q, k, v] tile views [128, S]
            stages = []  # deferred softmax normalizations
            if "t" in parts:
                excons = const.tile([1, 4], f32, name="excons")

            def ensure_qkv(b):
                if b not in qkvt:
                    sl = b % 2
                    qkvt[b] = [q_st[:, sl, :], k_st[:, sl, :], v_st[:, sl, :]]

            def a_units(n):
                """Filler units for 512-wide qkv chunk n (0..15): per m-tile,
                two 4-MM units; the second adds the bias TSP. The xc DMA is
                issued by the first unit."""
                b, nl = divmod(n, NG)
                ensure_qkv(b)
                cs = slice(nl * 512, (nl + 1) * 512)
                state = {}

                def dma_unit():
                    xc = xp.tile([128, KT_E, 512], bf16, tag="xc", name=f"xc{n}")
                    nc.sync.dma_start(
                        out=xc, in_=xT_r[:, :, n * 512:(n + 1) * 512])
                    state["xc"] = xc

                def mm_unit(m, half):
                    if half == 0:
                        state[m] = psfl.tile([128, 512], f32, tag="fl",
                                             name=f"qps{n}{m}")
                    ps = state[m]
                    for kt in range(4 * half, 4 * half + 4):
                        nc.tensor.matmul(
                            ps, lhsT=wq_sb[:, kt, m * 128:(m + 1) * 128],
                            rhs=state["xc"][:, kt, :],
                            start=(kt == 0), stop=(kt == KT_E - 1))
                    if half == 1:
                        nc.vector.tensor_scalar_add(
                            qkvt[b][m][:, cs], ps, bq_sb[:, m:m + 1])

                units = [dma_unit]
                for m in range(3):
                    units.append(lambda m=m: mm_unit(m, 0))
                    units.append(lambda m=m: mm_unit(m, 1))
                return units

            def vt_units(b):
                """16 transpose+copy units for batch b's v -> vk tiles."""
                sl = b % 2

                def unit(kt):
                    vt = psfl.tile([128, HPC, 64], f32r, tag="fl", name="vt")
                    nc.tensor.transpose(
                        vt, in_=qkvt[b][2][:, kt * 128:(kt + 1) * 128],
                        identity=id_sb)
                    nc.vector.tensor_copy(vk_st[:, sl, kt, :, 0:64], vt)

                return [lambda kt=kt: unit(kt) for kt in range(KT_S)]

            def o_units(b, ab, late):
                """16 out-projection units for batch b: (o-tile, quarter) with
                qt in {0,1} (early: reads ab groups 0-1) or {2,3} (late)."""
                state = {}
                qts = (2, 3) if late else (0, 1)

                def unit(o, qt):
                    if qt % 2 == 0:
                        state[o] = ystp.tile([128, 1024], bf16, tag="yst",
                                             name=f"yst{o}{qt}")
                    yp = psfl.tile([128, 512], f32, tag="fl", name="yp")
                    nc.tensor.matmul(
                        yp, lhsT=wo_sb[:, o * 128:(o + 1) * 128],
                        rhs=ab[:, qt * 512:(qt + 1) * 512],
                        start=True, stop=True)
                    nc.vector.tensor_scalar_add(
                        state[o][:, (qt % 2) * 512:(qt % 2 + 1) * 512], yp,
                        bo_sb[:, o:o + 1])
                    if qt % 2 == 1:
                        nc.sync.dma_start(
                            out=yT[o * 128:(o + 1) * 128,
                                   b * S + (qt // 2) * 1024:
                                   b * S + (qt // 2 + 1) * 1024],
                            in_=state[o])

                return [lambda o=o, qt=qt: unit(o, qt)
                        for o in range(E // 128) for qt in qts]

            def emit_attn_group(b, c, ab, fillers):
                """Both heads for 512-wide sq chunk c; pops one filler unit
                per kt slot."""
                skip_at = "t" in parts
                sl = b % 2
                cq = c * 512
                at = [] if skip_at else [
                    psat.tile([65, 512], f32, tag="at", name=f"at{b}{c}{h}")
                    for h in range(HPC)]

                def emit_at(kt, ex):
                    for h in range(HPC):
                        nc.tensor.matmul(
                            at[h][:, :],
                            lhsT=vk_st[:, sl, kt, h, :],
                            rhs=ex[:, h * 512:(h + 1) * 512],
                            start=(kt == 0), stop=(kt == KT_S - 1))

                while len(stages) > 2:
                    emit_normalize()
                pending = []
                for kt in range(KT_S):
                    ks = slice(kt * 128, (kt + 1) * 128)
                    # emit ready at-MMs first: sc MMs below may stall on the
                    # score-slot sem and would head-block them on in-order PE
                    if not skip_at and len(pending) >= 2:
                        emit_at(*pending.pop(0))
                    scp = pssc.tile([128, HPC, 512], f32, tag="sc", name="scp")
                    for h in range(HPC):
                        hp = slice(64 * h, 64 * (h + 1))
                        nc.tensor.matmul(
                            scp[:, h, :],
                            lhsT=k_st[hp, sl, ks],
                            rhs=q_st[hp, sl, cq:cq + 512],
                            start=True, stop=True,
                            tile_position=(64 * h, 0))
                    ex = expp.tile([128, 1024], bf16, tag="exp")
                    nc.scalar.activation(ex, scp, EXP, scale=0.125)
                    if skip_at:
                        nc.vector.tensor_copy(excons, ex[0:1, 0:8].bitcast(f32))
                    else:
                        pending.append((kt, ex))
                    if fillers:
                        fillers.pop(0)()
                if skip_at:
                    return
                for kp, exp_ in pending:
                    emit_at(kp, exp_)
                # evacuate PSUM accumulators immediately (frees psat slots for
                # the next group); normalize later from SBUF, off-critical-path
                for h in range(HPC):
                    st = anp.tile([65, 512], f32, tag="stg", name=f"stg{h}")
                    nc.vector.tensor_copy(st, at[h])
                    rs = anp.tile([1, 512], f32, tag="rsn", name="rs")
                    nc.sync.dma_start(out=rs, in_=st[64:65, :])
                    stages.append((st, rs, ab, cq, h))

            def emit_normalize():
                st, rs, ab, cq, h = stages.pop(0)
                rr = anp.tile([1, 512], f32, tag="rrn", name="rr")
                nc.vector.reciprocal(rr, rs)
                rb = anp.tile([64, 512], f32, tag="rbn", name="rb")
                nc.gpsimd.partition_broadcast(rb, rr)
                if h == 0:
                    nc.vector.tensor_mul(
                        ab[0:64, cq:cq + 512], st[0:64, :], rb)
                else:
                    nm = anp.tile([64, 512], bf16, tag="normb", name="nm")
                    nc.vector.tensor_mul(nm, st[0:64, :], rb)
                    nc.sync.dma_start(
                        out=ab[64:128, cq:cq + 512], in_=nm)

            # ---- batch 0 qkv upfront ----
            for n in range(NG):
                for u in a_units(n):
                    u()
            abs_ = {}
            for b in range(B):
                abs_[b] = None if ("t" in parts) else asp.tile(
                    [128, S], bf16, tag="ab", name=f"ab{b}")
                fillers = []
                if "a" in parts:
                    fillers += vt_units(b)
                mids = []
                if b + 1 < B:
                    for n in range(NG * (b + 1), NG * (b + 2)):
                        mids += a_units(n)
                if b >= 1 and "o" in parts:
                    # weave out-proj of b-1 between qkv units of b+1
                    late = (o_units(b - 1, abs_[b - 1], False)
                            + o_units(b - 1, abs_[b - 1], True))
                    w = []
                    while mids or late:
                        if mids:
                            w.append(mids.pop(0))
                        if late:
                            w.append(late.pop(0))
                    mids = w
                fillers += mids
                if "a" in parts:
                    for c in range(NG):
                        emit_attn_group(b, c, abs_[b], fillers)
                for u in fillers:
                    u()
                if b == B - 1:
                    while stages:
                        emit_normalize()
                    if "o" in parts:
                        for u in (o_units(b, abs_[b], False)
                                  + o_units(b, abs_[b], True)):
                            u()
                if niter is not None and parts != "Aao" and "o" not in parts:
                    cons_b = const.tile([1, 4], f32, name=f"cons{b}", bufs=1) \
                        if b == 0 else cons_b
                    for t in range(2):
                        nc.vector.tensor_copy(
                            cons_b, qkvt[b][t][0:1, 0:8].bitcast(f32))
                    nc.vector.tensor_copy(
                        cons_b, qkvt[b][2][0:1, 0:4].bitcast(f32))
                    if "a" in parts and "t" not in parts:
                        nc.vector.tensor_copy(
                            cons_b, abs_[b][0:1, 0:8].bitcast(f32))


        if niter is None:
            body()
        else:
            with tc.For_i(0, niter, 1):
                body()
            dmy = const.tile([1, 3], f32)
            nc.vector.tensor_copy(dmy, bq_sb[0:1, 0:3])
            nc.gpsimd.dma_start(out=tout[:, :], in_=dmy)

    nc.compile()
    _prog_cache[key] = nc
    return nc


def make_in_maps(x, W_qkv, b_qkv, W_out, b_out):
    xT = np.ascontiguousarray(x.reshape(BS, E).T).astype(NPBF16)
    in_maps = []
    for c in range(NCORES):
        rows, brows = [], []
        for blk in range(3):
            for h in (HPC * c, HPC * c + 1):
                rows.append(W_qkv[blk * E + h * D: blk * E + (h + 1) * D, :])
                brows.append(b_qkv[blk * E + h * D: blk * E + (h + 1) * D])
        W_loc = np.concatenate(rows, axis=0)            # [384, 1024]
        b_loc = np.concatenate(brows, axis=0)           # [384]
        wq_in = np.ascontiguousarray(W_loc.T).astype(NPBF16)
        bq_in = np.ascontiguousarray(b_loc.reshape(3, 128).T).astype(np.float32)
        wo_in = np.ascontiguousarray(
            W_out[:, c * F:(c + 1) * F].T).astype(NPBF16)
        if c == 0:
            bo_in = np.ascontiguousarray(
                b_out.reshape(E // 128, 128).T).astype(np.float32)
        else:
            bo_in = np.zeros((128, E // 128), dtype=np.float32)
        in_maps.append(
            {"xT": xT, "wq": wq_in, "bq": bq_in, "wo": wo_in, "bo": bo_in})
    return in_maps


def kernel(x, W_qkv, b_qkv, W_out, b_out):
    x = np.asarray(x, dtype=np.float32)
    W_qkv = np.asarray(W_qkv, dtype=np.float32)
    b_qkv = np.asarray(b_qkv, dtype=np.float32)
    W_out = np.asarray(W_out, dtype=np.float32)
    b_out = np.asarray(b_out, dtype=np.float32)

    nc = build_program()
    in_maps = make_in_maps(x, W_qkv, b_qkv, W_out, b_out)
    res = run_bass_kernel_spmd(nc, in_maps, core_ids=list(range(NCORES)))
    acc = np.zeros((E, BS), dtype=np.float32)
    for c in range(NCORES):
        acc += res.results[c]["yT"].astype(np.float32)
    return np.ascontiguousarray(acc.T).reshape(B, S, E)


if __name__ == "__main__":
    rng = np.random.default_rng(0)
    x = rng.standard_normal((B, S, E), dtype=np.float32)
    s = 1.0 / np.sqrt(E)
    W_qkv = rng.uniform(-s, s, (3 * E, E)).astype(np.float32)
    b_qkv = rng.uniform(-s, s, (3 * E,)).astype(np.float32)
    W_out = rng.uniform(-s, s, (E, E)).astype(np.float32)
    b_out = rng.uniform(-s, s, (E,)).astype(np.float32)
    y = kernel(x, W_qkv, b_qkv, W_out, b_out)
    print("out", y.shape, y.dtype, float(np.abs(y).max()))


# revision 3
# speedup vs baseline: 1.0964x; 1.0964x over previous
"""Multi-head attention (B=4, S=2048, E=1024, H=16, D=64) on 8 TRN2 cores.

Sharding: heads 2c, 2c+1 on core c (Megatron-style column-parallel qkv,
row-parallel out-projection; partial outputs summed on host in f32).

v2 vs baseline: all matmul inputs bf16 (PSUM accumulation stays f32, rel err
~4e-3 vs 2e-2 budget) halving input/output DMA, and two structural changes:

1. Row-packed score matmuls: the two heads' K=64 score contractions run
   CONCURRENTLY in the PE array via tile_position=(0,0)/(64,0) (HW-measured
   2.56x over the baseline's zero-padded K=128 trick).
2. The softmax exp stream paces the kernel (ScalarE is the busiest engine:
   256 x exp[128,1024] ~ 290us). Attention runs on 512-wide sq chunks with a
   combined head-pair score tile [128,2,512] (2 PSUM banks) so a 2-deep sc
   pipeline (4 banks) + per-head attnV accumulators (2 banks) + a filler pool
   (2 banks) fit in the 8 PSUM banks. One exp instruction per kt covers both
   heads; the sc pipeline keeps ScalarE saturated while qkv / out-projection /
   v-transpose work is drip-fed to the in-order PE as fine-grained filler
   units (one per kt slot) from a per-batch queue.

attnV keeps the appended ones-column on v producing softmax denominators as
PSUM row 64 (the two heads' v cannot be column-packed: 2x65 outputs > 128
array columns); normalization is reciprocal + partition-broadcast + multiply.
"""
from contextlib import ExitStack

import numpy as np

import concourse.bass as bass
import concourse.mybir as mybir
import concourse.tile as tile
from concourse import bacc
from concourse.bass_utils import run_bass_kernel_spmd
from concourse.masks import make_identity

B, S, E, H, D = 4, 2048, 1024, 16, 64
NCORES = 8
HPC = H // NCORES        # 2 heads per core
F = HPC * D              # 128 local features
M3 = 3 * F               # 384 local qkv rows
BS = B * S               # 8192
KT_E = E // 128          # 8 contraction tiles for projections
KT_S = S // 128          # 16 sk tiles
NG = S // 512            # 4 attention groups (sq chunks) per batch
f32 = mybir.dt.float32
f32r = mybir.dt.float32r
bf16 = mybir.dt.bfloat16
EXP = mybir.ActivationFunctionType.Exp
NPBF16 = mybir.dt.np(bf16)

_prog_cache = {}


def build_program(niter=None, parts="Aao"):
    """niter=None: normal external-I/O program. niter=N: timing variant with
    internal DRAM x/y and the whole body in a device-side For_i loop."""
    key = ("nc", niter, parts)
    if key in _prog_cache:
        return _prog_cache[key]
    nc = bacc.Bacc("TRN2", target_bir_lowering=False)
    if niter is None:
        xT = nc.dram_tensor("xT", [E, BS], bf16, kind="ExternalInput")
        yT = nc.dram_tensor("yT", [E, BS], bf16, kind="ExternalOutput")
    else:
        xT = nc.dram_tensor("xTi", [E, BS], bf16, kind="Internal")
        yT = nc.dram_tensor("yTi", [E, BS], bf16, kind="Internal")
    wq = nc.dram_tensor("wq", [E, M3], bf16, kind="ExternalInput")
    bq = nc.dram_tensor("bq", [128, 3], f32, kind="ExternalInput")
    wo = nc.dram_tensor("wo", [F, E], bf16, kind="ExternalInput")
    bo = nc.dram_tensor("bo", [128, E // 128], f32, kind="ExternalInput")
    if niter is not None:
        tout = nc.dram_tensor("tout", [1, 3], f32, kind="ExternalOutput")

    with tile.TileContext(nc) as tc, ExitStack() as ctx:
        const = ctx.enter_context(tc.tile_pool(name="const", bufs=1))
        xp = ctx.enter_context(tc.tile_pool(name="xp", bufs=2))
        expp = ctx.enter_context(tc.tile_pool(name="expp", bufs=4))
        anp = ctx.enter_context(tc.tile_pool(name="anp", bufs=4))
        asp = ctx.enter_context(tc.tile_pool(name="asp", bufs=2))
        ystp = ctx.enter_context(tc.tile_pool(name="ystp", bufs=3))
        # PSUM (8 banks): pssc 2x[128,2,512] sc-pairs (4), psat 2x[65,512]
        # attnV accumulators (2), psfl 2x[128,512] filler slots (2).
        pssc = ctx.enter_context(tc.tile_pool(name="pssc", bufs=2, space="PSUM"))
        psat = ctx.enter_context(tc.tile_pool(name="psat", bufs=2, space="PSUM"))
        psfl = ctx.enter_context(tc.tile_pool(name="psfl", bufs=2, space="PSUM"))

        wq_sb = const.tile([128, KT_E, M3], bf16)
        nc.gpsimd.dma_start(out=wq_sb, in_=wq.rearrange("(kt p) m -> p kt m", p=128))
        wo_sb = const.tile([F, E], bf16)
        nc.gpsimd.dma_start(out=wo_sb, in_=wo[:, :])
        bq_sb = const.tile([128, 3], f32)
        nc.gpsimd.dma_start(out=bq_sb, in_=bq[:, :])
        bo_sb = const.tile([128, E // 128], f32)
        nc.gpsimd.dma_start(out=bo_sb, in_=bo[:, :])
        id_f32 = const.tile([128, 128], f32)
        make_identity(nc, id_f32)
        id_sb = const.tile([128, 128], f32r)
        nc.vector.tensor_copy(id_sb, id_f32)

        xT_r = xT.rearrange("(kt p) n -> p kt n", p=128)

        # persistent double-buffered qkv storage, slot b%2.
        q_st = const.tile([128, 2, S], bf16, name="q_st")
        k_st = const.tile([128, 2, S], bf16, name="k_st")
        v_st = const.tile([128, 2, S], f32r, name="v_st")
        # vk: [part, slot, kt, head, 65] bf16; col 64 = ones (denominator).
        vk_st = const.tile([128, 2, KT_S, HPC, 65], bf16, name="vk_st")
        nc.vector.memset(vk_st[:, :, :, :, 64:65], 1.0)
        # dead allocation, kept deliberately: it offsets the SBUF addresses
        # of everything allocated after it, which measures ~120us faster
        # (SBUF subbank conflict luck). Do not remove without re-timing.
        pad_st = const.tile([128, 2, S], bf16, name="pad_st")

        def body():
            qkvt = {}   # batch -> [q, k, v] tile views [128, S]
            stages = []  # deferred softmax normalizations
            if "t" in parts:
                excons = const.tile([1, 4], f32, name="excons")

            def ensure_qkv(b):
                if b not in qkvt:
                    sl = b % 2
                    qkvt[b] = [q_st[:, sl, :], k_st[:, sl, :], v_st[:, sl, :]]

            def a_units(n):
                """Filler units for 512-wide qkv chunk n (0..15): per m-tile,
                two 4-MM units; the second adds the bias TSP. The xc DMA is
                issued by the first unit."""
                b, nl = divmod(n, NG)
                ensure_qkv(b)
                cs = slice(nl * 512, (nl + 1) * 512)
                state = {}

                def dma_unit():
                    xc = xp.tile([128, KT_E, 512], bf16, tag="xc", name=f"xc{n}")
                    nc.sync.dma_start(
                        out=xc, in_=xT_r[:, :, n * 512:(n + 1) * 512])
                    state["xc"] = xc

                def mm_unit(m, half):
                    if half == 0:
                        state[m] = psfl.tile([128, 512], f32, tag="fl",
                                             name=f"qps{n}{m}")
                    ps = state[m]
                    for kt in range(4 * half, 4 * half + 4):
                        nc.tensor.matmul(
                            ps, lhsT=wq_sb[:, kt, m * 128:(m + 1) * 128],
                            rhs=state["xc"][:, kt, :],
                            start=(kt == 0), stop=(kt == KT_E - 1))
                    if half == 1:
                        nc.vector.tensor_scalar_add(
                            qkvt[b][m][:, cs], ps, bq_sb[:, m:m + 1])

                units = [dma_unit]
                for m in range(3):
                    units.append(lambda m=m: mm_unit(m, 0))
                    units.append(lambda m=m: mm_unit(m, 1))
                return units

            def vt_units(b):
                """16 transpose+copy units for batch b's v -> vk tiles."""
                sl = b % 2

                def unit(kt):
                    vt = psfl.tile([128, HPC, 64], f32r, tag="fl", name="vt")
                    nc.tensor.transpose(
                        vt, in_=qkvt[b][2][:, kt * 128:(kt + 1) * 128],
                        identity=id_sb)
                    nc.vector.tensor_copy(vk_st[:, sl, kt, :, 0:64], vt)

                return [lambda kt=kt: unit(kt) for kt in range(KT_S)]

            def o_units(b, ab, late):
                """16 out-projection units for batch b: (o-tile, quarter) with
                qt in {0,1} (early: reads ab groups 0-1) or {2,3} (late)."""
                state = {}
                qts = (2, 3) if late else (0, 1)

                def unit(o, qt):
                    if qt % 2 == 0:
                        state[o] = ystp.tile([128, 1024], bf16, tag="yst",
                                             name=f"yst{o}{qt}")
                    yp = psfl.tile([128, 512], f32, tag="fl", name="yp")
                    nc.tensor.matmul(
                        yp, lhsT=wo_sb[:, o * 128:(o + 1) * 128],
                        rhs=ab[:, qt * 512:(qt + 1) * 512],
                        start=True, stop=True)
                    nc.vector.tensor_scalar_add(
                        state[o][:, (qt % 2) * 512:(qt % 2 + 1) * 512], yp,
                        bo_sb[:, o:o + 1])
                    if qt % 2 == 1:
                        nc.sync.dma_start(
                            out=yT[o * 128:(o + 1) * 128,
                                   b * S + (qt // 2) * 1024:
                                   b * S + (qt // 2 + 1) * 1024],
                            in_=state[o])

                return [lambda o=o, qt=qt: unit(o, qt)
                        for o in range(E // 128) for qt in qts]

            def emit_attn_group(b, c, ab, fillers):
                """Both heads for 512-wide sq chunk c; pops one filler unit
                per kt slot."""
                skip_at = "t" in parts
                sl = b % 2
                cq = c * 512
                at = [] if skip_at else [
                    psat.tile([65, 512], f32, tag="at", name=f"at{b}{c}{h}")
                    for h in range(HPC)]

                def emit_at(kt, ex):
                    for h in range(HPC):
                        nc.tensor.matmul(
                            at[h][:, :],
                            lhsT=vk_st[:, sl, kt, h, :],
                            rhs=ex[:, h * 512:(h + 1) * 512],
                            start=(kt == 0), stop=(kt == KT_S - 1))

                while len(stages) > 2:
                    emit_normalize()
                pending = []
                for kt in range(KT_S):
                    ks = slice(kt * 128, (kt + 1) * 128)
                    # emit ready at-MMs first: sc MMs below may stall on the
                    # score-slot sem and would head-block them on in-order PE
                    if not skip_at and len(pending) >= 2:
                        emit_at(*pending.pop(0))
                    scp = pssc.tile([128, HPC, 512], f32, tag="sc", name="scp")
                    for h in range(HPC):
                        hp = slice(64 * h, 64 * (h + 1))
                        nc.tensor.matmul(
                            scp[:, h, :],
                            lhsT=k_st[hp, sl, ks],
                            rhs=q_st[hp, sl, cq:cq + 512],
                            start=True, stop=True,
                            tile_position=(64 * h, 0))
                    ex = expp.tile([128, 1024], bf16, tag="exp")
                    nc.scalar.activation(ex, scp, EXP, scale=0.125)
                    if skip_at:
                        nc.vector.tensor_copy(excons, ex[0:1, 0:8].bitcast(f32))
                    else:
                        pending.append((kt, ex))
                    if fillers:
                        fillers.pop(0)()
                if skip_at:
                    return
                for kp, exp_ in pending:
                    emit_at(kp, exp_)
                # evacuate PSUM accumulators immediately (frees psat slots for
                # the next group); normalize later from SBUF, off-critical-path
                for h in range(HPC):
                    st = anp.tile([65, 512], f32, tag="stg", name=f"stg{h}")
                    nc.vector.tensor_copy(st, at[h])
                    rs = anp.tile([1, 512], f32, tag="rsn", name="rs")
                    nc.sync.dma_start(out=rs, in_=st[64:65, :])
                    stages.append((st, rs, ab, cq, h))

            def emit_normalize():
                st, rs, ab, cq, h = stages.pop(0)
                rr = anp.tile([1, 512], f32, tag="rrn", name="rr")
                nc.vector.reciprocal(rr, rs)
                rb = anp.tile([64, 512], f32, tag="rbn", name="rb")
                nc.gpsimd.partition_broadcast(rb, rr)
                if h == 0:
                    nc.vector.tensor_mul(
                        ab[0:64, cq:cq + 512], st[0:64, :], rb)
                else:
                    nm = anp.tile([64, 512], bf16, tag="normb", name="nm")
                    nc.vector.tensor_mul(nm, st[0:64, :], rb)
                    nc.sync.dma_start(
                        out=ab[64:128, cq:cq + 512], in_=nm)

            # ---- batch 0 qkv upfront ----
            for n in range(NG):
                for u in a_units(n):
                    u()
            abs_ = {}
            for b in range(B):
                abs_[b] = None if ("t" in parts) else asp.tile(
                    [128, S], bf16, tag="ab", name=f"ab{b}")
                fillers = []
                if "a" in parts:
                    fillers += vt_units(b)
                mids = []
                if b + 1 < B:
                    for n in range(NG * (b + 1), NG * (b + 2)):
                        mids += a_units(n)
                if b >= 1 and "o" in parts:
                    # weave out-proj of b-1 between qkv units of b+1
                    late = (o_units(b - 1, abs_[b - 1], False)
                            + o_units(b - 1, abs_[b - 1], True))
                    w = []
                    while mids or late:
                        if mids:
                            w.append(mids.pop(0))
                        if late:
                            w.append(late.pop(0))
                    mids = w
                fillers += mids
                if "a" in parts:
                    for c in range(NG):
                        emit_attn_group(b, c, abs_[b], fillers)
                for u in fillers:
                    u()
                if b == B - 1:
                    while stages:
                        emit_normalize()
                    if "o" in parts:
                        for u in (o_units(b, abs_[b], False)
                                  + o_units(b, abs_[b], True)):
                            u()
                if niter is not None and parts != "Aao" and "o" not in parts:
                    cons_b = const.tile([1, 4], f32, name=f"cons{b}", bufs=1) \
                        if b == 0 else cons_b
                    for t in range(2):
                        nc.vector.tensor_copy(
                            cons_b, qkvt[b][t][0:1, 0:8].bitcast(f32))
                    nc.vector.tensor_copy(
                        cons_b, qkvt[b][2][0:1, 0:4].bitcast(f32))
                    if "a" in parts and "t" not in parts:
                        nc.vector.tensor_copy(
                            cons_b, abs_[b][0:1, 0:8].bitcast(f32))


        if niter is None:
            body()
        else:
            with tc.For_i(0, niter, 1):
                body()
            dmy = const.tile([1, 3], f32)
            nc.vector.tensor_copy(dmy, bq_sb[0:1, 0:3])
            nc.gpsimd.dma_start(out=tout[:, :], in_=dmy)

    nc.compile()
    _prog_cache[key] = nc
    return nc


def make_in_maps(x, W_qkv, b_qkv, W_out, b_out):
    xT = np.ascontiguousarray(x.reshape(BS, E).T).astype(NPBF16)
    in_maps = []
    for c in range(NCORES):
        rows, brows = [], []
        for blk in range(3):
            for h in (HPC * c, HPC * c + 1):
                rows.append(W_qkv[blk * E + h * D: blk * E + (h + 1) * D, :])
                brows.append(b_qkv[blk * E + h * D: blk * E + (h + 1) * D])
        W_loc = np.concatenate(rows, axis=0)            # [384, 1024]
        b_loc = np.concatenate(brows, axis=0)           # [384]
        wq_in = np.ascontiguousarray(W_loc.T).astype(NPBF16)
        bq_in = np.ascontiguousarray(b_loc.reshape(3, 128).T).astype(np.float32)
        wo_in = np.ascontiguousarray(
            W_out[:, c * F:(c + 1) * F].T).astype(NPBF16)
        if c == 0:
            bo_in = np.ascontiguousarray(
                b_out.reshape(E // 128, 128).T).astype(np.float32)
        else:
            bo_in = np.zeros((128, E // 128), dtype=np.float32)
        in_maps.append(
            {"xT": xT, "wq": wq_in, "bq": bq_in, "wo": wo_in, "bo": bo_in})
    return in_maps


def kernel(x, W_qkv, b_qkv, W_out, b_out):
    x = np.asarray(x, dtype=np.float32)
    W_qkv = np.asarray(W_qkv, dtype=np.float32)
    b_qkv = np.asarray(b_qkv, dtype=np.float32)
    W_out = np.asarray(W_out, dtype=np.float32)
    b_out = np.asarray(b_out, dtype=np.float32)

    nc = build_program()
    in_maps = make_in_maps(x, W_qkv, b_qkv, W_out, b_out)
    res = run_bass_kernel_spmd(nc, in_maps, core_ids=list(range(NCORES)))
    acc = np.zeros((E, BS), dtype=np.float32)
    for c in range(NCORES):
        acc += res.results[c]["yT"].astype(np.float32)
    return np.ascontiguousarray(acc.T).reshape(B, S, E)


if __name__ == "__main__":
    rng = np.random.default_rng(0)
    x = rng.standard_normal((B, S, E), dtype=np.float32)
    s = 1.0 / np.sqrt(E)
    W_qkv = rng.uniform(-s, s, (3 * E, E)).astype(np.float32)
    b_qkv = rng.uniform(-s, s, (3 * E,)).astype(np.float32)
    W_out = rng.uniform(-s, s, (E, E)).astype(np.float32)
    b_out = rng.uniform(-s, s, (E,)).astype(np.float32)
    y = kernel(x, W_qkv, b_qkv, W_out, b_out)
    print("out", y.shape, y.dtype, float(np.abs(y).max()))
